# revision 34
# baseline (speedup 1.0000x reference)
"""Additive (Bahdanau) attention on 8 TRN2 NeuronCores.

Math per batch b (masked positions contribute exactly 0 after exp):
    q = queries[b] @ W_q              (Q, H)
    k = keys[b]    @ W_k              (K, H)
    S[i, j] = sum_h w_v[h] * tanh(q[i,h] + k[j,h])
    out[b]  = softmax_j(S masked) @ values[b]

Sharding: the mask is a prefix mask (positions >= valid_len are dead), so
only sum(valid_lens) key columns carry work.  The host splits each batch's
valid-key prefix into jobs of KJ keys and hands each of the 8 cores NJ=2
jobs.  A job scores its KJ keys against all Q queries of its batch and
emits unnormalized partials (O^T = sum_j e^S v_j, l = sum_j e^S); the host
sums partials per batch and divides.  No max-subtraction is needed:
|S| <= sum|w_v| ~ 7, so exp never overflows in f32.

Device pipeline per key pair (2j, 2j+1), h in partitions:
  DVE  presum[:, :] = q2 + k2[:, j]      (tensor_scalar, per-partition addend)
  ACT  feats = tanh(presum)              (bulk over GP pairs, bf16 out)
  PE   S^T[2t:2t+2, :] += wpat_t^T @ feats   (zero-padded stationary lands
       each pair's scores at the right PSUM partitions -> S^T in key order)
  ACT  P = exp(S^T + mask_bias)          (prefix mask rides the bias)
  PE   O^T += V^T_block @ P ; l += 1^T @ P
"""

import sys

sys.path.insert(0, "/opt/trn_rl_repo")

import numpy as np

B, Q, KLEN, D_IN, H, D_V = 4, 1024, 1024, 256, 64, 128
NCORES = 8
NJ = 2  # jobs per core
MASK_VAL = -1.0e6
GP = 10  # key-pairs per bulk-tanh group

_CACHE = {}
LAST_RESULT = None


def _group_sizes(npairs, ramp_up, ramp_down):
    """Bulk-tanh group sizes: mostly GP, with small lead-in/lead-out groups at
    the kernel boundaries so ACT/PE pipeline fill+drain don't serialize (and
    the PE never idles past the HAM re-throttle window at the tail)."""
    up = [1, 1, 2, 4] if ramp_up else []
    down = [4, 2, 1, 1] if ramp_down else []
    mid = npairs - sum(up) - sum(down)
    if mid < 0:
        return [(2, "act")] * (npairs // 2) + [(1, "act")] * (npairs % 2)
    sizes = up + [GP] * (mid // GP) + ([mid % GP] if mid % GP else []) + down
    plan = [(s, "act") for s in sizes]
    if ramp_up:
        # first two lead-in groups skip the DVE presum (bias-fused tanh):
        # shortens the kernel-start critical chain by the DVE hop
        plan[0] = (plan[0][0], "actb")
        plan[1] = (plan[1][0], "actb")
    return plan


def _plan(vl):
    """Choose job size KJ and split batches' valid prefixes into NCORES*NJ jobs."""
    nslots = NCORES * NJ
    kj = 32
    while sum(-(-v // kj) for v in vl) > nslots:
        kj += 32
    jobs = []  # (batch, start, cnt)
    for b, v in enumerate(vl):
        nb_jobs = -(-v // kj)
        base, rem = divmod(v, nb_jobs)
        s = 0
        for i in range(nb_jobs):
            cnt = base + (1 if i < rem else 0)
            jobs.append((b, s, cnt))
            s += cnt
    while len(jobs) < nslots:
        jobs.append((0, 0, 0))  # empty padding job
    return kj, jobs


def _build(kj, repeat=1):
    import concourse.tile as tile
    from concourse import bacc, mybir

    fp32 = mybir.dt.float32
    bf16 = mybir.dt.bfloat16
    Tanh = mybir.ActivationFunctionType.Tanh
    Exp = mybir.ActivationFunctionType.Exp
    nbj = -(-kj // 128)  # key blocks per job
    hKJ = kj // 2

    nc = bacc.Bacc(
        "TRN2", target_bir_lowering=False, debug=False, num_devices=NCORES
    )
    qtsE = nc.dram_tensor("qts", [128, NJ * 2 * Q], bf16, kind="ExternalInput").ap()
    ktsE = nc.dram_tensor("kts", [128, NJ * 2 * kj], bf16, kind="ExternalInput").ap()
    vallE = nc.dram_tensor(
        "vall", [128, NJ * nbj * D_V], bf16, kind="ExternalInput"
    ).ap()
    mRE = nc.dram_tensor("maskR", [128, NJ * nbj], fp32, kind="ExternalInput").ap()
    wqE = nc.dram_tensor("wq", [128, 2 * H], bf16, kind="ExternalInput").ap()
    wkE = nc.dram_tensor("wk", [128, 2 * H], bf16, kind="ExternalInput").ap()
    wpE = nc.dram_tensor("wpat", [128, 512], bf16, kind="ExternalInput").ap()
    outE = nc.dram_tensor("out", [NJ * (D_V + 1), Q], fp32, kind="ExternalOutput").ap()

    with tile.TileContext(nc) as tc:
        with (
            tc.tile_pool(name="const", bufs=1) as cp,
            tc.tile_pool(name="feats", bufs=2) as fpool,
            tc.tile_pool(name="probs", bufs=2) as prp,
            tc.tile_pool(name="ps1", bufs=1, space="PSUM") as ps1,
            tc.tile_pool(name="ps2", bufs=2, space="PSUM") as ps2,
        ):
            # --- input DMAs: one contiguous transfer per tensor, spread over
            # both HWDGE rings (sync, scalar) + SWDGE (gpsimd); q-side first so
            # projections start early.
            qts = cp.tile([128, NJ * 2 * Q], bf16)
            for jn in range(NJ):
                nc.sync.dma_start(
                    qts[:, jn * 2 * Q : (jn + 1) * 2 * Q],
                    qtsE[:, jn * 2 * Q : (jn + 1) * 2 * Q],
                )
            wq_sb = cp.tile([128, 2 * H], bf16)
            nc.scalar.dma_start(wq_sb[:], wqE[:, :])
            wk_sb = cp.tile([128, 2 * H], bf16)
            nc.scalar.dma_start(wk_sb[:], wkE[:, :])
            kts = cp.tile([128, NJ * 2 * kj], bf16)
            nc.scalar.dma_start(kts[:], ktsE[:, :])
            wp_sb = cp.tile([128, 512], bf16)
            nc.gpsimd.dma_start(wp_sb[:], wpE[:, :])
            mask_sb = cp.tile([128, NJ * nbj], fp32)
            nc.gpsimd.dma_start(mask_sb[:], mRE[:, :])
            vall = cp.tile([128, NJ * nbj * D_V], bf16)
            nc.gpsimd.dma_start(vall[:], vallE[:, :])
            ones_sb = cp.tile([128, 1], bf16)
            nc.vector.memset(ones_sb[:], 1.0)

            o_sb = cp.tile([128, NJ * Q], fp32, name="o_sb")
            lo_sb = cp.tile([1, NJ * Q], fp32, name="lo_sb")

            def emit_proj(rep, jn):
                """Project one job's queries/keys; returns (q2, k2) SBUF tiles."""
                qof = jn * 2 * Q
                # q_proj^T: qh halves stacked in partitions of one bank
                qproj_ps = ps2.tile(
                    [128, 512], fp32, tag="S0", name=f"qproj_{rep}_{jn}", bufs=2
                )
                for qh in range(2):
                    for cc in range(2):
                        nc.tensor.matmul(
                            qproj_ps[64 * qh : 64 * qh + 64, :],
                            wq_sb[:, cc * H : (cc + 1) * H],
                            qts[:, qof + cc * Q + qh * 512 : qof + cc * Q + qh * 512 + 512],
                            start=(cc == 0),
                            stop=(cc == 1),
                        )
                q2_sb = cp.tile([128, Q], bf16, tag=f"q2_{jn}", name=f"q2_{rep}_{jn}")
                for qh in range(2):
                    if qh == 0 or jn > 0:
                        nc.vector.tensor_copy(
                            q2_sb[0:64, qh * 512 : qh * 512 + 512],
                            qproj_ps[64 * qh : 64 * qh + 64, :],
                        )
                        nc.vector.tensor_copy(
                            q2_sb[64:128, qh * 512 : qh * 512 + 512],
                            qproj_ps[64 * qh : 64 * qh + 64, :],
                        )
                    else:
                        nc.scalar.copy(
                            q2_sb[0:64, qh * 512 : qh * 512 + 512],
                            qproj_ps[64 * qh : 64 * qh + 64, :],
                        )
                        nc.scalar.copy(
                            q2_sb[64:128, qh * 512 : qh * 512 + 512],
                            qproj_ps[64 * qh : 64 * qh + 64, :],
                        )
                # k2 = paired key projections: [:64] even keys, [64:] odd
                kof = jn * 2 * kj
                kproj_ps = ps2.tile(
                    [128, 512], fp32, tag="S1", name=f"kproj_{rep}_{jn}", bufs=2
                )
                for half in range(2):
                    for cc in range(2):
                        nc.tensor.matmul(
                            kproj_ps[64 * half : 64 * half + 64, 0:hKJ],
                            wk_sb[:, cc * H : (cc + 1) * H],
                            kts[:, kof + cc * kj + half * hKJ : kof + cc * kj + half * hKJ + hKJ],
                            start=(cc == 0),
                            stop=(cc == 1),
                        )
                k2_sb = cp.tile(
                    [128, hKJ], fp32, tag=f"k2_{jn}", name=f"k2_{rep}_{jn}"
                )
                nc.vector.tensor_copy(k2_sb[:], kproj_ps[:, 0:hKJ])
                return q2_sb, k2_sb

            for rep in range(repeat):
                # job 0's projections immediately; job 1's are emitted after
                # job 0's ramp-up groups (lower scheduler priority) so the
                # first tanh isn't stuck behind job 1's DVE copies.
                proj = {0: emit_proj(rep, 0)}

                for jn in range(NJ):
                    O_ps = [
                        ps1.tile(
                            [128, 512], fp32, tag=f"O{qh}", name=f"O{qh}_{rep}_{jn}"
                        )
                        for qh in range(2)
                    ]
                    l_ps = [
                        ps1.tile(
                            [1, 512], fp32, tag=f"l{qh}", name=f"l{qh}_{rep}_{jn}"
                        )
                        for qh in range(2)
                    ]
                    q2_sb, k2_sb = proj[jn]

                    # main loop over 128-key blocks of this job
                    for m in range(nbj):
                        kb = min(128, kj - m * 128)
                        npair_m = kb // 2
                        S_ps = [
                            ps2.tile(
                                [128, 512],
                                fp32,
                                tag=f"S{qh}",
                                name=f"S{qh}_{rep}_{jn}_{m}",
                                bufs=2,
                            )
                            for qh in range(2)
                        ]
                        gsizes = _group_sizes(
                            npair_m,
                            ramp_up=(rep == 0 and jn == 0 and m == 0),
                            ramp_down=(
                                rep == repeat - 1 and jn == NJ - 1 and m == nbj - 1
                            ),
                        )
                        tp0 = 0
                        for grp, (gp, eng) in enumerate(gsizes):
                            feats = fpool.tile(
                                [128, gp * Q],
                                bf16,
                                name=f"feats_{rep}_{jn}_{m}_{grp}",
                                tag="feats",
                                bufs=4,
                            )
                            if eng == "actb":
                                for p in range(gp):
                                    j = 64 * m + tp0 + p
                                    nc.scalar.activation(
                                        feats[:, p * Q : (p + 1) * Q],
                                        q2_sb[:],
                                        Tanh,
                                        bias=k2_sb[:, j : j + 1],
                                        scale=1.0,
                                    )
                            else:
                                presum = fpool.tile(
                                    [128, gp * Q],
                                    bf16,
                                    name=f"presum_{rep}_{jn}_{m}_{grp}",
                                    tag="presum",
                                    bufs=4,
                                )
                                for p in range(gp):
                                    j = 64 * m + tp0 + p
                                    nc.vector.tensor_scalar_add(
                                        presum[:, p * Q : (p + 1) * Q],
                                        q2_sb[:],
                                        k2_sb[:, j : j + 1],
                                    )
                                nc.scalar.activation(
                                    feats[:, 0 : gp * Q], presum[:, 0 : gp * Q], Tanh
                                )
                            for p in range(gp):
                                tp = tp0 + p
                                g, tl = divmod(tp, 16)
                                for qh in range(2):
                                    nc.tensor.matmul(
                                        S_ps[qh][32 * g : 32 * g + 32, :],
                                        wp_sb[:, 32 * tl : 32 * tl + 32],
                                        feats[:, p * Q + qh * 512 : p * Q + qh * 512 + 512],
                                        start=(tl == 0),
                                        stop=(tl == 15 or tp == npair_m - 1),
                                        tile_position=(0, 32 * g),
                                    )
                            tp0 += gp
                            if (
                                jn == 0
                                and m == 0
                                and grp == 6
                                and NJ > 1
                                and (jn + 1) not in proj
                            ):
                                proj[jn + 1] = emit_proj(rep, jn + 1)
                        if jn == 0 and m == 0 and NJ > 1 and 1 not in proj:
                            proj[1] = emit_proj(rep, 1)
                        for qh in range(2):
                            P_sb = prp.tile(
                                [128, 512],
                                bf16,
                                tag=f"P{qh}",
                                name=f"P{qh}_{rep}_{jn}_{m}",
                                bufs=2,
                            )
                            nc.scalar.activation(
                                P_sb[0:kb, :],
                                S_ps[qh][0:kb, :],
                                Exp,
                                bias=mask_sb[0:kb, jn * nbj + m : jn * nbj + m + 1],
                                scale=1.0,
                            )
                            nc.tensor.matmul(
                                O_ps[qh][:],
                                vall[0:kb, (jn * nbj + m) * D_V : (jn * nbj + m + 1) * D_V],
                                P_sb[0:kb, :],
                                start=(m == 0),
                                stop=(m == nbj - 1),
                            )
                            nc.tensor.matmul(
                                l_ps[qh][:],
                                ones_sb[0:kb, :],
                                P_sb[0:kb, :],
                                start=(m == 0),
                                stop=(m == nbj - 1),
                            )

                    for qh in range(2):
                        nc.vector.tensor_copy(
                            o_sb[:, jn * Q + qh * 512 : jn * Q + qh * 512 + 512],
                            O_ps[qh][:],
                        )
                        nc.vector.tensor_copy(
                            lo_sb[:, jn * Q + qh * 512 : jn * Q + qh * 512 + 512],
                            l_ps[qh][:],
                        )
                    if rep == repeat - 1:
                        nc.sync.dma_start(
                            outE[jn * (D_V + 1) : jn * (D_V + 1) + D_V, :],
                            o_sb[:, jn * Q : (jn + 1) * Q],
                        )
                        nc.sync.dma_start(
                            outE[jn * (D_V + 1) + D_V : jn * (D_V + 1) + D_V + 1, :],
                            lo_sb[:, jn * Q : (jn + 1) * Q],
                        )

    nc.compile()
    return nc


def _prepare(inputs):
    import ml_dtypes

    bf16 = ml_dtypes.bfloat16
    queries = np.asarray(inputs["queries"], dtype=np.float32)
    keys = np.asarray(inputs["keys"], dtype=np.float32)
    values = np.asarray(inputs["values"], dtype=np.float32)
    valid_lens = np.asarray(inputs["valid_lens"]).astype(np.int64)
    W_q = np.asarray(inputs["W_q"], dtype=np.float32)
    W_k = np.asarray(inputs["W_k"], dtype=np.float32)
    w_v = np.asarray(inputs["w_v"], dtype=np.float32)

    kj, jobs = _plan([int(x) for x in valid_lens])
    nbj = -(-kj // 128)

    wpat = np.zeros((128, 512), np.float32)
    for t in range(16):
        wpat[0:64, 32 * t + 2 * t] = w_v
        wpat[64:128, 32 * t + 2 * t + 1] = w_v
    wpat = wpat.astype(bf16)
    wq_r = np.concatenate([W_q[0:128], W_q[128:256]], axis=1).astype(bf16)
    wk_r = np.concatenate([W_k[0:128], W_k[128:256]], axis=1).astype(bf16)

    qT = {b: np.ascontiguousarray(queries[b].T) for b in range(B)}

    in_maps = []
    for c in range(NCORES):
        qts = np.empty((128, NJ * 2 * Q), bf16)
        kts = np.empty((128, NJ * 2 * kj), bf16)
        vall = np.zeros((128, NJ * nbj * D_V), bf16)
        maskR = np.full((128, NJ * nbj), MASK_VAL, np.float32)
        for jn in range(NJ):
            b, s, cnt = jobs[c * NJ + jn]
            qts[:, jn * 2 * Q : jn * 2 * Q + Q] = qT[b][0:128].astype(bf16)
            qts[:, jn * 2 * Q + Q : (jn + 1) * 2 * Q] = qT[b][128:256].astype(bf16)
            kp = np.zeros((kj, D_IN), np.float32)
            kp[0:cnt] = keys[b, s : s + cnt]
            kre = np.concatenate([kp[0::2], kp[1::2]], axis=0).T  # (256, kj)
            kts[:, jn * 2 * kj : jn * 2 * kj + kj] = kre[0:128].astype(bf16)
            kts[:, jn * 2 * kj + kj : (jn + 1) * 2 * kj] = kre[128:256].astype(bf16)
            vp = np.zeros((kj, D_V), np.float32)
            vp[0:cnt] = values[b, s : s + cnt]
            for m in range(nbj):
                kb = min(128, kj - m * 128)
                vall[0:kb, (jn * nbj + m) * D_V : (jn * nbj + m) * D_V + D_V] = vp[
                    m * 128 : m * 128 + kb
                ].astype(bf16)
                mm = np.full((128,), MASK_VAL, np.float32)
                nvalid = min(max(cnt - m * 128, 0), 128)
                mm[0:nvalid] = 0.0
                maskR[:, jn * nbj + m] = mm
        in_maps.append(
            {
                "qts": qts,
                "kts": kts,
                "vall": vall,
                "maskR": maskR,
                "wq": wq_r,
                "wk": wk_r,
                "wpat": wpat,
            }
        )
    return kj, jobs, in_maps


def kernel(**inputs):
    global LAST_RESULT
    kj, jobs, in_maps = _prepare(inputs)

    if kj not in _CACHE:
        _CACHE[kj] = _build(kj)
    nc = _CACHE[kj]

    from concourse.bass_utils import run_bass_kernel_spmd

    res = run_bass_kernel_spmd(nc, in_maps, core_ids=list(range(NCORES)))
    LAST_RESULT = res

    O = np.zeros((B, D_V, Q), np.float64)
    L = np.zeros((B, Q), np.float64)
    for c in range(NCORES):
        o = np.asarray(res.results[c]["out"])  # (NJ*(D_V+1), Q)
        for jn in range(NJ):
            b, s, cnt = jobs[c * NJ + jn]
            if cnt == 0:
                continue
            O[b] += o[jn * (D_V + 1) : jn * (D_V + 1) + D_V].astype(np.float64)
            L[b] += o[jn * (D_V + 1) + D_V].astype(np.float64)
    out = (O / L[:, None, :]).transpose(0, 2, 1)
    return np.ascontiguousarray(out.astype(np.float32))


# revision 35
# speedup vs baseline: 1.2722x; 1.2722x over previous
"""Additive (Bahdanau) attention on 8 TRN2 NeuronCores.

Math per batch b (masked positions contribute exactly 0 after exp):
    q = queries[b] @ W_q              (Q, H)
    k = keys[b]    @ W_k              (K, H)
    S[i, j] = sum_h w_v[h] * tanh(q[i,h] + k[j,h])
    out[b]  = softmax_j(S masked) @ values[b]

Sharding: the mask is a prefix mask (positions >= valid_len are dead), so
only sum(valid_lens) key columns carry work.  The host splits each batch's
valid-key prefix into jobs of KJ keys and hands each of the 8 cores NJ=2
jobs.  A job scores its KJ keys against all Q queries of its batch and
emits unnormalized partials (O^T = sum_j e^S v_j, l = sum_j e^S); the host
sums partials per batch and divides.  No max-subtraction is needed:
|S| <= sum|w_v| ~ 7, so exp never overflows in f32.

Device pipeline per key pair (2j, 2j+1), h in partitions:
  DVE  presum[:, :] = q2 + k2[:, j]      (tensor_scalar, per-partition addend)
  ACT  feats = tanh(presum)              (bulk over GP pairs, bf16 out)
  PE   S^T[2t:2t+2, :] += wpat_t^T @ feats   (zero-padded stationary lands
       each pair's scores at the right PSUM partitions -> S^T in key order)
  ACT  P = exp(S^T + mask_bias)          (prefix mask rides the bias)
  PE   O^T += V^T_block @ P ; l += 1^T @ P
"""

import sys

sys.path.insert(0, "/opt/trn_rl_repo")

import numpy as np

B, Q, KLEN, D_IN, H, D_V = 4, 1024, 1024, 256, 64, 128
NCORES = 8
NJ = 2  # jobs per core
MASK_VAL = -1.0e6
GP = 10  # key-pairs per bulk-tanh group

_CACHE = {}
LAST_RESULT = None


def _group_sizes(npairs, ramp_up, ramp_down):
    """Bulk-tanh group sizes: mostly GP, with small lead-in/lead-out groups at
    the kernel boundaries so ACT/PE pipeline fill+drain don't serialize (and
    the PE never idles past the HAM re-throttle window at the tail)."""
    up = [1, 1, 2, 4] if ramp_up else []
    down = [4, 2, 1, 1] if ramp_down else []
    mid = npairs - sum(up) - sum(down)
    if mid < 0:
        return [(2, "act")] * (npairs // 2) + [(1, "act")] * (npairs % 2)
    sizes = up + [GP] * (mid // GP) + ([mid % GP] if mid % GP else []) + down
    plan = [(s, "act") for s in sizes]
    if ramp_up:
        # first two lead-in groups skip the DVE presum (bias-fused tanh):
        # shortens the kernel-start critical chain by the DVE hop
        plan[0] = (plan[0][0], "actb")
        plan[1] = (plan[1][0], "actb")
    return plan


def _plan(vl):
    """Choose job size KJ and split batches' valid prefixes into NCORES*NJ jobs."""
    nslots = NCORES * NJ
    kj = 32
    while sum(-(-v // kj) for v in vl) > nslots:
        kj += 32
    jobs = []  # (batch, start, cnt)
    for b, v in enumerate(vl):
        nb_jobs = -(-v // kj)
        base, rem = divmod(v, nb_jobs)
        s = 0
        for i in range(nb_jobs):
            cnt = base + (1 if i < rem else 0)
            jobs.append((b, s, cnt))
            s += cnt
    while len(jobs) < nslots:
        jobs.append((0, 0, 0))  # empty padding job
    return kj, jobs


def _build(kj, repeat=1):
    import concourse.tile as tile
    from concourse import bacc, mybir

    fp32 = mybir.dt.float32
    bf16 = mybir.dt.bfloat16
    Tanh = mybir.ActivationFunctionType.Tanh
    Exp = mybir.ActivationFunctionType.Exp
    nbj = -(-kj // 128)  # key blocks per job
    hKJ = kj // 2

    nc = bacc.Bacc(
        "TRN2", target_bir_lowering=False, debug=False, num_devices=NCORES
    )
    qtsE = nc.dram_tensor("qts", [128, NJ * 2 * Q], bf16, kind="ExternalInput").ap()
    ktsE = nc.dram_tensor("kts", [128, NJ * 2 * kj], bf16, kind="ExternalInput").ap()
    vallE = nc.dram_tensor(
        "vall", [128, NJ * nbj * D_V], bf16, kind="ExternalInput"
    ).ap()
    mRE = nc.dram_tensor("maskR", [128, NJ * nbj], fp32, kind="ExternalInput").ap()
    wqE = nc.dram_tensor("wq", [128, 2 * H], bf16, kind="ExternalInput").ap()
    wkE = nc.dram_tensor("wk", [128, 2 * H], bf16, kind="ExternalInput").ap()
    wpE = nc.dram_tensor("wpat", [128, 512], bf16, kind="ExternalInput").ap()
    outE = nc.dram_tensor("out", [NJ * (D_V + 1), Q], fp32, kind="ExternalOutput").ap()

    with tile.TileContext(nc) as tc:
        with (
            tc.tile_pool(name="const", bufs=1) as cp,
            tc.tile_pool(name="feats", bufs=2) as fpool,
            tc.tile_pool(name="probs", bufs=2) as prp,
            tc.tile_pool(name="ps1", bufs=1, space="PSUM") as ps1,
            tc.tile_pool(name="ps2", bufs=2, space="PSUM") as ps2,
        ):
            # --- input DMAs: one contiguous transfer per tensor, spread over
            # both HWDGE rings (sync, scalar) + SWDGE (gpsimd); q-side first so
            # projections start early.
            qts = cp.tile([128, NJ * 2 * Q], bf16)
            for jn in range(NJ):
                nc.sync.dma_start(
                    qts[:, jn * 2 * Q : (jn + 1) * 2 * Q],
                    qtsE[:, jn * 2 * Q : (jn + 1) * 2 * Q],
                )
            wq_sb = cp.tile([128, 2 * H], bf16)
            nc.scalar.dma_start(wq_sb[:], wqE[:, :])
            wk_sb = cp.tile([128, 2 * H], bf16)
            nc.scalar.dma_start(wk_sb[:], wkE[:, :])
            kts = cp.tile([128, NJ * 2 * kj], bf16)
            nc.scalar.dma_start(kts[:], ktsE[:, :])
            wp_sb = cp.tile([128, 512], bf16)
            nc.gpsimd.dma_start(wp_sb[:], wpE[:, :])
            mask_sb = cp.tile([128, NJ * nbj], fp32)
            nc.gpsimd.dma_start(mask_sb[:], mRE[:, :])
            vall = cp.tile([128, NJ * nbj * D_V], bf16)
            nc.gpsimd.dma_start(vall[:], vallE[:, :])
            ones_sb = cp.tile([128, 1], bf16)
            nc.vector.memset(ones_sb[:], 1.0)

            o_sb = cp.tile([128, NJ * Q], fp32, name="o_sb")
            lo_sb = cp.tile([1, NJ * Q], fp32, name="lo_sb")

            def emit_proj(rep, jn):
                """Project one job's queries/keys; returns (q2, k2) SBUF tiles."""
                qof = jn * 2 * Q
                # q_proj^T: qh halves stacked in partitions of one bank
                qproj_ps = ps2.tile(
                    [128, 512], fp32, tag="S0", name=f"qproj_{rep}_{jn}", bufs=2
                )
                for qh in range(2):
                    for cc in range(2):
                        nc.tensor.matmul(
                            qproj_ps[64 * qh : 64 * qh + 64, :],
                            wq_sb[:, cc * H : (cc + 1) * H],
                            qts[:, qof + cc * Q + qh * 512 : qof + cc * Q + qh * 512 + 512],
                            start=(cc == 0),
                            stop=(cc == 1),
                        )
                q2_sb = cp.tile([128, Q], bf16, tag=f"q2_{jn}", name=f"q2_{rep}_{jn}")
                for qh in range(2):
                    if qh == 0 or jn > 0 or rep > 0:
                        nc.vector.tensor_copy(
                            q2_sb[0:64, qh * 512 : qh * 512 + 512],
                            qproj_ps[64 * qh : 64 * qh + 64, :],
                        )
                        nc.vector.tensor_copy(
                            q2_sb[64:128, qh * 512 : qh * 512 + 512],
                            qproj_ps[64 * qh : 64 * qh + 64, :],
                        )
                    else:
                        nc.scalar.copy(
                            q2_sb[0:64, qh * 512 : qh * 512 + 512],
                            qproj_ps[64 * qh : 64 * qh + 64, :],
                        )
                        nc.scalar.copy(
                            q2_sb[64:128, qh * 512 : qh * 512 + 512],
                            qproj_ps[64 * qh : 64 * qh + 64, :],
                        )
                # k2 = paired key projections: [:64] even keys, [64:] odd
                kof = jn * 2 * kj
                kproj_ps = ps2.tile(
                    [128, 512], fp32, tag="S1", name=f"kproj_{rep}_{jn}", bufs=2
                )
                for half in range(2):
                    for cc in range(2):
                        nc.tensor.matmul(
                            kproj_ps[64 * half : 64 * half + 64, 0:hKJ],
                            wk_sb[:, cc * H : (cc + 1) * H],
                            kts[:, kof + cc * kj + half * hKJ : kof + cc * kj + half * hKJ + hKJ],
                            start=(cc == 0),
                            stop=(cc == 1),
                        )
                k2_sb = cp.tile(
                    [128, hKJ], fp32, tag=f"k2_{jn}", name=f"k2_{rep}_{jn}"
                )
                nc.vector.tensor_copy(k2_sb[:], kproj_ps[:, 0:hKJ])
                return q2_sb, k2_sb

            for rep in range(repeat):
                # job 0's projections immediately; job 1's are emitted after
                # job 0's ramp-up groups (lower scheduler priority) so the
                # first tanh isn't stuck behind job 1's DVE copies.
                proj = {0: emit_proj(rep, 0)}

                for jn in range(NJ):
                    O_ps = [
                        ps1.tile(
                            [128, 512], fp32, tag=f"O{qh}", name=f"O{qh}_{rep}_{jn}"
                        )
                        for qh in range(2)
                    ]
                    l_ps = [
                        ps1.tile(
                            [1, 512], fp32, tag=f"l{qh}", name=f"l{qh}_{rep}_{jn}"
                        )
                        for qh in range(2)
                    ]
                    q2_sb, k2_sb = proj[jn]

                    # main loop over 128-key blocks of this job
                    for m in range(nbj):
                        kb = min(128, kj - m * 128)
                        npair_m = kb // 2
                        S_ps = [
                            ps2.tile(
                                [128, 512],
                                fp32,
                                tag=f"S{qh}",
                                name=f"S{qh}_{rep}_{jn}_{m}",
                                bufs=2,
                            )
                            for qh in range(2)
                        ]
                        gsizes = _group_sizes(
                            npair_m,
                            ramp_up=(rep == 0 and jn == 0 and m == 0),
                            ramp_down=(
                                rep == repeat - 1 and jn == NJ - 1 and m == nbj - 1
                            ),
                        )
                        tp0 = 0
                        for grp, (gp, eng) in enumerate(gsizes):
                            feats = fpool.tile(
                                [128, gp * Q],
                                bf16,
                                name=f"feats_{rep}_{jn}_{m}_{grp}",
                                tag="feats",
                                bufs=4,
                            )
                            if eng == "actb":
                                for p in range(gp):
                                    j = 64 * m + tp0 + p
                                    nc.scalar.activation(
                                        feats[:, p * Q : (p + 1) * Q],
                                        q2_sb[:],
                                        Tanh,
                                        bias=k2_sb[:, j : j + 1],
                                        scale=1.0,
                                    )
                            else:
                                presum = fpool.tile(
                                    [128, gp * Q],
                                    bf16,
                                    name=f"presum_{rep}_{jn}_{m}_{grp}",
                                    tag="presum",
                                    bufs=4,
                                )
                                for p in range(gp):
                                    j = 64 * m + tp0 + p
                                    nc.vector.tensor_scalar_add(
                                        presum[:, p * Q : (p + 1) * Q],
                                        q2_sb[:],
                                        k2_sb[:, j : j + 1],
                                    )
                                nc.scalar.activation(
                                    feats[:, 0 : gp * Q], presum[:, 0 : gp * Q], Tanh
                                )
                            for p in range(gp):
                                tp = tp0 + p
                                g, tl = divmod(tp, 16)
                                for qh in range(2):
                                    nc.tensor.matmul(
                                        S_ps[qh][32 * g : 32 * g + 32, :],
                                        wp_sb[:, 32 * tl : 32 * tl + 32],
                                        feats[:, p * Q + qh * 512 : p * Q + qh * 512 + 512],
                                        start=(tl == 0),
                                        stop=(tl == 15 or tp == npair_m - 1),
                                        tile_position=(0, 32 * g),
                                    )
                            tp0 += gp
                            if (
                                jn == 0
                                and m == 0
                                and grp == 6
                                and NJ > 1
                                and (jn + 1) not in proj
                            ):
                                proj[jn + 1] = emit_proj(rep, jn + 1)
                        if jn == 0 and m == 0 and NJ > 1 and 1 not in proj:
                            proj[1] = emit_proj(rep, 1)
                        for qh in range(2):
                            P_sb = prp.tile(
                                [128, 512],
                                bf16,
                                tag=f"P{qh}",
                                name=f"P{qh}_{rep}_{jn}_{m}",
                                bufs=2,
                            )
                            nc.scalar.activation(
                                P_sb[0:kb, :],
                                S_ps[qh][0:kb, :],
                                Exp,
                                bias=mask_sb[0:kb, jn * nbj + m : jn * nbj + m + 1],
                                scale=1.0,
                            )
                            nc.tensor.matmul(
                                O_ps[qh][:],
                                vall[0:kb, (jn * nbj + m) * D_V : (jn * nbj + m + 1) * D_V],
                                P_sb[0:kb, :],
                                start=(m == 0),
                                stop=(m == nbj - 1),
                            )
                            nc.tensor.matmul(
                                l_ps[qh][:],
                                ones_sb[0:kb, :],
                                P_sb[0:kb, :],
                                start=(m == 0),
                                stop=(m == nbj - 1),
                            )

                    for qh in range(2):
                        nc.vector.tensor_copy(
                            o_sb[:, jn * Q + qh * 512 : jn * Q + qh * 512 + 512],
                            O_ps[qh][:],
                        )
                        nc.vector.tensor_copy(
                            lo_sb[:, jn * Q + qh * 512 : jn * Q + qh * 512 + 512],
                            l_ps[qh][:],
                        )
                    if rep == repeat - 1:
                        nc.sync.dma_start(
                            outE[jn * (D_V + 1) : jn * (D_V + 1) + D_V, :],
                            o_sb[:, jn * Q : (jn + 1) * Q],
                        )
                        nc.sync.dma_start(
                            outE[jn * (D_V + 1) + D_V : jn * (D_V + 1) + D_V + 1, :],
                            lo_sb[:, jn * Q : (jn + 1) * Q],
                        )

    nc.compile()
    return nc


def _prepare(inputs):
    import ml_dtypes

    bf16 = ml_dtypes.bfloat16
    queries = np.asarray(inputs["queries"], dtype=np.float32)
    keys = np.asarray(inputs["keys"], dtype=np.float32)
    values = np.asarray(inputs["values"], dtype=np.float32)
    valid_lens = np.asarray(inputs["valid_lens"]).astype(np.int64)
    W_q = np.asarray(inputs["W_q"], dtype=np.float32)
    W_k = np.asarray(inputs["W_k"], dtype=np.float32)
    w_v = np.asarray(inputs["w_v"], dtype=np.float32)

    kj, jobs = _plan([int(x) for x in valid_lens])
    nbj = -(-kj // 128)

    wpat = np.zeros((128, 512), np.float32)
    for t in range(16):
        wpat[0:64, 32 * t + 2 * t] = w_v
        wpat[64:128, 32 * t + 2 * t + 1] = w_v
    wpat = wpat.astype(bf16)
    wq_r = np.concatenate([W_q[0:128], W_q[128:256]], axis=1).astype(bf16)
    wk_r = np.concatenate([W_k[0:128], W_k[128:256]], axis=1).astype(bf16)

    qT = {b: np.ascontiguousarray(queries[b].T) for b in range(B)}

    in_maps = []
    for c in range(NCORES):
        qts = np.empty((128, NJ * 2 * Q), bf16)
        kts = np.empty((128, NJ * 2 * kj), bf16)
        vall = np.zeros((128, NJ * nbj * D_V), bf16)
        maskR = np.full((128, NJ * nbj), MASK_VAL, np.float32)
        for jn in range(NJ):
            b, s, cnt = jobs[c * NJ + jn]
            qts[:, jn * 2 * Q : jn * 2 * Q + Q] = qT[b][0:128].astype(bf16)
            qts[:, jn * 2 * Q + Q : (jn + 1) * 2 * Q] = qT[b][128:256].astype(bf16)
            kp = np.zeros((kj, D_IN), np.float32)
            kp[0:cnt] = keys[b, s : s + cnt]
            kre = np.concatenate([kp[0::2], kp[1::2]], axis=0).T  # (256, kj)
            kts[:, jn * 2 * kj : jn * 2 * kj + kj] = kre[0:128].astype(bf16)
            kts[:, jn * 2 * kj + kj : (jn + 1) * 2 * kj] = kre[128:256].astype(bf16)
            vp = np.zeros((kj, D_V), np.float32)
            vp[0:cnt] = values[b, s : s + cnt]
            for m in range(nbj):
                kb = min(128, kj - m * 128)
                vall[0:kb, (jn * nbj + m) * D_V : (jn * nbj + m) * D_V + D_V] = vp[
                    m * 128 : m * 128 + kb
                ].astype(bf16)
                mm = np.full((128,), MASK_VAL, np.float32)
                nvalid = min(max(cnt - m * 128, 0), 128)
                mm[0:nvalid] = 0.0
                maskR[:, jn * nbj + m] = mm
        in_maps.append(
            {
                "qts": qts,
                "kts": kts,
                "vall": vall,
                "maskR": maskR,
                "wq": wq_r,
                "wk": wk_r,
                "wpat": wpat,
            }
        )
    return kj, jobs, in_maps


def kernel(**inputs):
    global LAST_RESULT
    kj, jobs, in_maps = _prepare(inputs)

    if kj not in _CACHE:
        _CACHE[kj] = _build(kj)
    nc = _CACHE[kj]

    from concourse.bass_utils import run_bass_kernel_spmd

    res = run_bass_kernel_spmd(nc, in_maps, core_ids=list(range(NCORES)))
    LAST_RESULT = res

    O = np.zeros((B, D_V, Q), np.float64)
    L = np.zeros((B, Q), np.float64)
    for c in range(NCORES):
        o = np.asarray(res.results[c]["out"])  # (NJ*(D_V+1), Q)
        for jn in range(NJ):
            b, s, cnt = jobs[c * NJ + jn]
            if cnt == 0:
                continue
            O[b] += o[jn * (D_V + 1) : jn * (D_V + 1) + D_V].astype(np.float64)
            L[b] += o[jn * (D_V + 1) + D_V].astype(np.float64)
    out = (O / L[:, None, :]).transpose(0, 2, 1)
    return np.ascontiguousarray(out.astype(np.float32))


# revision 37
# speedup vs baseline: 1.3767x; 1.0821x over previous
"""Additive (Bahdanau) attention on 8 TRN2 NeuronCores.

Math per batch b (masked positions contribute exactly 0 after exp):
    q = queries[b] @ W_q              (Q, H)
    k = keys[b]    @ W_k              (K, H)
    S[i, j] = sum_h w_v[h] * tanh(q[i,h] + k[j,h])
    out[b]  = softmax_j(S masked) @ values[b]

Sharding: the mask is a prefix mask (positions >= valid_len are dead), so
only sum(valid_lens) key columns carry work.  The host splits each batch's
valid-key prefix into jobs of KJ keys and hands each of the 8 cores NJ=2
jobs.  A job scores its KJ keys against all Q queries of its batch and
emits unnormalized partials (O^T = sum_j e^S v_j, l = sum_j e^S); the host
sums partials per batch and divides.  No max-subtraction is needed:
|S| <= sum|w_v| ~ 7, so exp never overflows in f32.

Device pipeline per key pair (2j, 2j+1), h in partitions:
  DVE  presum[:, :] = q2 + k2[:, j]      (tensor_scalar, per-partition addend)
  ACT  feats = tanh(presum)              (bulk over GP pairs, bf16 out)
  PE   S^T[2t:2t+2, :] += wpat_t^T @ feats   (zero-padded stationary lands
       each pair's scores at the right PSUM partitions -> S^T in key order)
  ACT  P = exp(S^T + mask_bias)          (prefix mask rides the bias)
  PE   O^T += V^T_block @ P ; l += 1^T @ P
"""

import sys

sys.path.insert(0, "/opt/trn_rl_repo")

import numpy as np

B, Q, KLEN, D_IN, H, D_V = 4, 1024, 1024, 256, 64, 128
NCORES = 8
NJ = 2  # jobs per core
MASK_VAL = -1.0e6
GP = 10  # key-pairs per bulk-tanh group

_CACHE = {}
LAST_RESULT = None


def _group_sizes(npairs, ramp_up, ramp_down):
    """Bulk-tanh group sizes: mostly GP, with small lead-in/lead-out groups at
    the kernel boundaries so ACT/PE pipeline fill+drain don't serialize (and
    the PE never idles past the HAM re-throttle window at the tail)."""
    up = [1, 1, 2, 4] if ramp_up else []
    down = [4, 2, 1, 1] if ramp_down else []
    mid = npairs - sum(up) - sum(down)
    if mid < 0:
        return [(2, "act")] * (npairs // 2) + [(1, "act")] * (npairs % 2)
    sizes = up + [GP] * (mid // GP) + ([mid % GP] if mid % GP else []) + down
    plan = []
    # 2-pair DVE-path (tanh addition formula) groups per block; fewer in the
    # final (ramp-down) block where the thinning pipeline can't hide them
    ndve = 2 if ramp_down else 4
    for s in sizes:
        if s == GP and ndve > 0:
            plan.append((s - 2, "act"))
            plan.append((2, "dve"))
            ndve -= 1
        else:
            plan.append((s, "act"))
    if ramp_up:
        # first two lead-in groups skip the DVE presum (bias-fused tanh):
        # shortens the kernel-start critical chain by the DVE hop
        plan[0] = (plan[0][0], "actb")
        plan[1] = (plan[1][0], "actb")
    assert sum(s for s, _ in plan) == npairs
    return plan


def _plan(vl):
    """Choose job size KJ and split batches' valid prefixes into NCORES*NJ jobs."""
    nslots = NCORES * NJ
    kj = 32
    while sum(-(-v // kj) for v in vl) > nslots:
        kj += 32
    jobs = []  # (batch, start, cnt)
    for b, v in enumerate(vl):
        nb_jobs = -(-v // kj)
        base, rem = divmod(v, nb_jobs)
        s = 0
        for i in range(nb_jobs):
            cnt = base + (1 if i < rem else 0)
            jobs.append((b, s, cnt))
            s += cnt
    while len(jobs) < nslots:
        jobs.append((0, 0, 0))  # empty padding job
    return kj, jobs


def _build(kj, repeat=1):
    import concourse.tile as tile
    from concourse import bacc, mybir

    fp32 = mybir.dt.float32
    bf16 = mybir.dt.bfloat16
    Tanh = mybir.ActivationFunctionType.Tanh
    Exp = mybir.ActivationFunctionType.Exp
    nbj = -(-kj // 128)  # key blocks per job
    hKJ = kj // 2

    nc = bacc.Bacc(
        "TRN2", target_bir_lowering=False, debug=False, num_devices=NCORES
    )
    qtsE = nc.dram_tensor("qts", [128, NJ * 2 * Q], bf16, kind="ExternalInput").ap()
    ktsE = nc.dram_tensor("kts", [128, NJ * 2 * kj], bf16, kind="ExternalInput").ap()
    vallE = nc.dram_tensor(
        "vall", [128, NJ * nbj * D_V], bf16, kind="ExternalInput"
    ).ap()
    mRE = nc.dram_tensor("maskR", [128, NJ * nbj], fp32, kind="ExternalInput").ap()
    wqE = nc.dram_tensor("wq", [128, 2 * H], bf16, kind="ExternalInput").ap()
    wkE = nc.dram_tensor("wk", [128, 2 * H], bf16, kind="ExternalInput").ap()
    wpE = nc.dram_tensor("wpat", [128, 512], bf16, kind="ExternalInput").ap()
    outE = nc.dram_tensor("out", [NJ * (D_V + 1), Q], fp32, kind="ExternalOutput").ap()

    with tile.TileContext(nc) as tc:
        with (
            tc.tile_pool(name="const", bufs=1) as cp,
            tc.tile_pool(name="feats", bufs=2) as fpool,
            tc.tile_pool(name="probs", bufs=2) as prp,
            tc.tile_pool(name="ps1", bufs=1, space="PSUM") as ps1,
            tc.tile_pool(name="ps2", bufs=2, space="PSUM") as ps2,
        ):
            # --- input DMAs: one contiguous transfer per tensor, spread over
            # both HWDGE rings (sync, scalar) + SWDGE (gpsimd); q-side first so
            # projections start early.
            qts = cp.tile([128, NJ * 2 * Q], bf16)
            for jn in range(NJ):
                nc.sync.dma_start(
                    qts[:, jn * 2 * Q : (jn + 1) * 2 * Q],
                    qtsE[:, jn * 2 * Q : (jn + 1) * 2 * Q],
                )
            wq_sb = cp.tile([128, 2 * H], bf16)
            nc.scalar.dma_start(wq_sb[:], wqE[:, :])
            wk_sb = cp.tile([128, 2 * H], bf16)
            nc.scalar.dma_start(wk_sb[:], wkE[:, :])
            kts = cp.tile([128, NJ * 2 * kj], bf16)
            nc.scalar.dma_start(kts[:], ktsE[:, :])
            wp_sb = cp.tile([128, 512], bf16)
            nc.gpsimd.dma_start(wp_sb[:], wpE[:, :])
            mask_sb = cp.tile([128, NJ * nbj], fp32)
            nc.gpsimd.dma_start(mask_sb[:], mRE[:, :])
            vall = cp.tile([128, NJ * nbj * D_V], bf16)
            nc.gpsimd.dma_start(vall[:], vallE[:, :])
            ones_sb = cp.tile([128, 1], bf16)
            nc.vector.memset(ones_sb[:], 1.0)

            o_sb = cp.tile([128, NJ * Q], fp32, name="o_sb")
            lo_sb = cp.tile([1, NJ * Q], fp32, name="lo_sb")

            def emit_proj(rep, jn):
                """Project one job's queries/keys; returns (q2, k2) SBUF tiles."""
                qof = jn * 2 * Q
                # q_proj^T: qh halves stacked in partitions of one bank
                qproj_ps = ps2.tile(
                    [128, 512], fp32, tag="S0", name=f"qproj_{rep}_{jn}", bufs=2
                )
                for qh in range(2):
                    for cc in range(2):
                        nc.tensor.matmul(
                            qproj_ps[64 * qh : 64 * qh + 64, :],
                            wq_sb[:, cc * H : (cc + 1) * H],
                            qts[:, qof + cc * Q + qh * 512 : qof + cc * Q + qh * 512 + 512],
                            start=(cc == 0),
                            stop=(cc == 1),
                        )
                q2_sb = cp.tile([128, Q], bf16, tag=f"q2_{jn}", name=f"q2_{rep}_{jn}")
                for qh in range(2):
                    if qh == 0 or jn > 0 or rep > 0:
                        nc.vector.tensor_copy(
                            q2_sb[0:64, qh * 512 : qh * 512 + 512],
                            qproj_ps[64 * qh : 64 * qh + 64, :],
                        )
                        nc.vector.tensor_copy(
                            q2_sb[64:128, qh * 512 : qh * 512 + 512],
                            qproj_ps[64 * qh : 64 * qh + 64, :],
                        )
                    else:
                        nc.scalar.copy(
                            q2_sb[0:64, qh * 512 : qh * 512 + 512],
                            qproj_ps[64 * qh : 64 * qh + 64, :],
                        )
                        nc.scalar.copy(
                            q2_sb[64:128, qh * 512 : qh * 512 + 512],
                            qproj_ps[64 * qh : 64 * qh + 64, :],
                        )
                # k2 = paired key projections: [:64] even keys, [64:] odd
                kof = jn * 2 * kj
                kproj_ps = ps2.tile(
                    [128, 512], fp32, tag="S1", name=f"kproj_{rep}_{jn}", bufs=2
                )
                for half in range(2):
                    for cc in range(2):
                        nc.tensor.matmul(
                            kproj_ps[64 * half : 64 * half + 64, 0:hKJ],
                            wk_sb[:, cc * H : (cc + 1) * H],
                            kts[:, kof + cc * kj + half * hKJ : kof + cc * kj + half * hKJ + hKJ],
                            start=(cc == 0),
                            stop=(cc == 1),
                        )
                k2_sb = cp.tile(
                    [128, hKJ], fp32, tag=f"k2_{jn}", name=f"k2_{rep}_{jn}"
                )
                nc.vector.tensor_copy(k2_sb[:], kproj_ps[:, 0:hKJ])
                return q2_sb, k2_sb

            for rep in range(repeat):
                # job 0's projections immediately; job 1's are emitted after
                # job 0's ramp-up groups (lower scheduler priority) so the
                # first tanh isn't stuck behind job 1's DVE copies.
                proj = {0: emit_proj(rep, 0)}

                for jn in range(NJ):
                    O_ps = [
                        ps1.tile(
                            [128, 512], fp32, tag=f"O{qh}", name=f"O{qh}_{rep}_{jn}"
                        )
                        for qh in range(2)
                    ]
                    l_ps = [
                        ps1.tile(
                            [1, 512], fp32, tag=f"l{qh}", name=f"l{qh}_{rep}_{jn}"
                        )
                        for qh in range(2)
                    ]
                    q2_sb, k2_sb = proj[jn]
                    tq2_sb = cp.tile(
                        [128, Q], fp32, tag=f"tq2_{jn}", name=f"tq2_{rep}_{jn}"
                    )
                    nc.scalar.activation(tq2_sb[:], q2_sb[:], Tanh)
                    tk2_sb = cp.tile(
                        [128, hKJ], fp32, tag=f"tk2_{jn}", name=f"tk2_{rep}_{jn}"
                    )
                    nc.scalar.activation(tk2_sb[:], k2_sb[:], Tanh)

                    # main loop over 128-key blocks of this job
                    for m in range(nbj):
                        kb = min(128, kj - m * 128)
                        npair_m = kb // 2
                        S_ps = [
                            ps2.tile(
                                [128, 512],
                                fp32,
                                tag=f"S{qh}",
                                name=f"S{qh}_{rep}_{jn}_{m}",
                                bufs=2,
                            )
                            for qh in range(2)
                        ]
                        gsizes = _group_sizes(
                            npair_m,
                            ramp_up=(rep == 0 and jn == 0 and m == 0),
                            ramp_down=(
                                rep == repeat - 1 and jn == NJ - 1 and m == nbj - 1
                            ),
                        )
                        tp0 = 0
                        for grp, (gp, eng) in enumerate(gsizes):
                            feats = fpool.tile(
                                [128, gp * Q],
                                bf16,
                                name=f"feats_{rep}_{jn}_{m}_{grp}",
                                tag="dfeats" if eng == "dve" else "feats",
                                bufs=2 if eng == "dve" else 3,
                            )
                            if eng == "dve":
                                # tanh(q+k) = (tq+tk)/(1+tq*tk), all on DVE
                                u_sb = fpool.tile(
                                    [128, gp * Q],
                                    fp32,
                                    name=f"u_{rep}_{jn}_{m}_{grp}",
                                    tag="upath",
                                    bufs=2,
                                )
                                for p in range(gp):
                                    j = 64 * m + tp0 + p
                                    nc.vector.tensor_scalar(
                                        u_sb[:, p * Q : (p + 1) * Q],
                                        tq2_sb[:],
                                        tk2_sb[:, j : j + 1],
                                        1.0,
                                        mybir.AluOpType.mult,
                                        mybir.AluOpType.add,
                                    )
                                nc.vector.reciprocal_approx_fast(
                                    u_sb[:, 0 : gp * Q], u_sb[:, 0 : gp * Q]
                                )
                                for p in range(gp):
                                    j = 64 * m + tp0 + p
                                    nc.vector.scalar_tensor_tensor(
                                        feats[:, p * Q : (p + 1) * Q],
                                        tq2_sb[:],
                                        tk2_sb[:, j : j + 1],
                                        u_sb[:, p * Q : (p + 1) * Q],
                                        mybir.AluOpType.add,
                                        mybir.AluOpType.mult,
                                    )
                            elif eng == "actb":
                                for p in range(gp):
                                    j = 64 * m + tp0 + p
                                    nc.scalar.activation(
                                        feats[:, p * Q : (p + 1) * Q],
                                        q2_sb[:],
                                        Tanh,
                                        bias=k2_sb[:, j : j + 1],
                                        scale=1.0,
                                    )
                            else:
                                presum = fpool.tile(
                                    [128, gp * Q],
                                    bf16,
                                    name=f"presum_{rep}_{jn}_{m}_{grp}",
                                    tag="presum",
                                    bufs=3,
                                )
                                for p in range(gp):
                                    j = 64 * m + tp0 + p
                                    nc.vector.tensor_scalar_add(
                                        presum[:, p * Q : (p + 1) * Q],
                                        q2_sb[:],
                                        k2_sb[:, j : j + 1],
                                    )
                                nc.scalar.activation(
                                    feats[:, 0 : gp * Q], presum[:, 0 : gp * Q], Tanh
                                )
                            for p in range(gp):
                                tp = tp0 + p
                                g, tl = divmod(tp, 16)
                                for qh in range(2):
                                    nc.tensor.matmul(
                                        S_ps[qh][32 * g : 32 * g + 32, :],
                                        wp_sb[:, 32 * tl : 32 * tl + 32],
                                        feats[:, p * Q + qh * 512 : p * Q + qh * 512 + 512],
                                        start=(tl == 0),
                                        stop=(tl == 15 or tp == npair_m - 1),
                                        tile_position=(0, 32 * g),
                                    )
                            tp0 += gp
                            if (
                                jn == 0
                                and m == 0
                                and grp == 6
                                and NJ > 1
                                and (jn + 1) not in proj
                            ):
                                proj[jn + 1] = emit_proj(rep, jn + 1)
                        if jn == 0 and m == 0 and NJ > 1 and 1 not in proj:
                            proj[1] = emit_proj(rep, 1)
                        for qh in range(2):
                            P_sb = prp.tile(
                                [128, 512],
                                bf16,
                                tag=f"P{qh}",
                                name=f"P{qh}_{rep}_{jn}_{m}",
                                bufs=2,
                            )
                            nc.scalar.activation(
                                P_sb[0:kb, :],
                                S_ps[qh][0:kb, :],
                                Exp,
                                bias=mask_sb[0:kb, jn * nbj + m : jn * nbj + m + 1],
                                scale=1.0,
                            )
                            nc.tensor.matmul(
                                O_ps[qh][:],
                                vall[0:kb, (jn * nbj + m) * D_V : (jn * nbj + m + 1) * D_V],
                                P_sb[0:kb, :],
                                start=(m == 0),
                                stop=(m == nbj - 1),
                            )
                            nc.tensor.matmul(
                                l_ps[qh][:],
                                ones_sb[0:kb, :],
                                P_sb[0:kb, :],
                                start=(m == 0),
                                stop=(m == nbj - 1),
                            )

                    for qh in range(2):
                        nc.vector.tensor_copy(
                            o_sb[:, jn * Q + qh * 512 : jn * Q + qh * 512 + 512],
                            O_ps[qh][:],
                        )
                        nc.vector.tensor_copy(
                            lo_sb[:, jn * Q + qh * 512 : jn * Q + qh * 512 + 512],
                            l_ps[qh][:],
                        )
                    if rep == repeat - 1:
                        nc.sync.dma_start(
                            outE[jn * (D_V + 1) : jn * (D_V + 1) + D_V, :],
                            o_sb[:, jn * Q : (jn + 1) * Q],
                        )
                        nc.sync.dma_start(
                            outE[jn * (D_V + 1) + D_V : jn * (D_V + 1) + D_V + 1, :],
                            lo_sb[:, jn * Q : (jn + 1) * Q],
                        )

    nc.compile()
    return nc


def _prepare(inputs):
    import ml_dtypes

    bf16 = ml_dtypes.bfloat16
    queries = np.asarray(inputs["queries"], dtype=np.float32)
    keys = np.asarray(inputs["keys"], dtype=np.float32)
    values = np.asarray(inputs["values"], dtype=np.float32)
    valid_lens = np.asarray(inputs["valid_lens"]).astype(np.int64)
    W_q = np.asarray(inputs["W_q"], dtype=np.float32)
    W_k = np.asarray(inputs["W_k"], dtype=np.float32)
    w_v = np.asarray(inputs["w_v"], dtype=np.float32)

    kj, jobs = _plan([int(x) for x in valid_lens])
    nbj = -(-kj // 128)

    wpat = np.zeros((128, 512), np.float32)
    for t in range(16):
        wpat[0:64, 32 * t + 2 * t] = w_v
        wpat[64:128, 32 * t + 2 * t + 1] = w_v
    wpat = wpat.astype(bf16)
    wq_r = np.concatenate([W_q[0:128], W_q[128:256]], axis=1).astype(bf16)
    wk_r = np.concatenate([W_k[0:128], W_k[128:256]], axis=1).astype(bf16)

    qT = {b: np.ascontiguousarray(queries[b].T) for b in range(B)}

    in_maps = []
    for c in range(NCORES):
        qts = np.empty((128, NJ * 2 * Q), bf16)
        kts = np.empty((128, NJ * 2 * kj), bf16)
        vall = np.zeros((128, NJ * nbj * D_V), bf16)
        maskR = np.full((128, NJ * nbj), MASK_VAL, np.float32)
        for jn in range(NJ):
            b, s, cnt = jobs[c * NJ + jn]
            qts[:, jn * 2 * Q : jn * 2 * Q + Q] = qT[b][0:128].astype(bf16)
            qts[:, jn * 2 * Q + Q : (jn + 1) * 2 * Q] = qT[b][128:256].astype(bf16)
            kp = np.zeros((kj, D_IN), np.float32)
            kp[0:cnt] = keys[b, s : s + cnt]
            kre = np.concatenate([kp[0::2], kp[1::2]], axis=0).T  # (256, kj)
            kts[:, jn * 2 * kj : jn * 2 * kj + kj] = kre[0:128].astype(bf16)
            kts[:, jn * 2 * kj + kj : (jn + 1) * 2 * kj] = kre[128:256].astype(bf16)
            vp = np.zeros((kj, D_V), np.float32)
            vp[0:cnt] = values[b, s : s + cnt]
            for m in range(nbj):
                kb = min(128, kj - m * 128)
                vall[0:kb, (jn * nbj + m) * D_V : (jn * nbj + m) * D_V + D_V] = vp[
                    m * 128 : m * 128 + kb
                ].astype(bf16)
                mm = np.full((128,), MASK_VAL, np.float32)
                nvalid = min(max(cnt - m * 128, 0), 128)
                mm[0:nvalid] = 0.0
                maskR[:, jn * nbj + m] = mm
        in_maps.append(
            {
                "qts": qts,
                "kts": kts,
                "vall": vall,
                "maskR": maskR,
                "wq": wq_r,
                "wk": wk_r,
                "wpat": wpat,
            }
        )
    return kj, jobs, in_maps


def kernel(**inputs):
    global LAST_RESULT
    kj, jobs, in_maps = _prepare(inputs)

    if kj not in _CACHE:
        _CACHE[kj] = _build(kj)
    nc = _CACHE[kj]

    from concourse.bass_utils import run_bass_kernel_spmd

    res = run_bass_kernel_spmd(nc, in_maps, core_ids=list(range(NCORES)))
    LAST_RESULT = res

    O = np.zeros((B, D_V, Q), np.float64)
    L = np.zeros((B, Q), np.float64)
    for c in range(NCORES):
        o = np.asarray(res.results[c]["out"])  # (NJ*(D_V+1), Q)
        for jn in range(NJ):
            b, s, cnt = jobs[c * NJ + jn]
            if cnt == 0:
                continue
            O[b] += o[jn * (D_V + 1) : jn * (D_V + 1) + D_V].astype(np.float64)
            L[b] += o[jn * (D_V + 1) + D_V].astype(np.float64)
    out = (O / L[:, None, :]).transpose(0, 2, 1)
    return np.ascontiguousarray(out.astype(np.float32))


# revision 44
# speedup vs baseline: 1.8753x; 1.3622x over previous
"""Additive (Bahdanau) attention on 8 TRN2 NeuronCores.

Math per batch b (masked positions contribute exactly 0 after exp):
    q = queries[b] @ W_q              (Q, H)
    k = keys[b]    @ W_k              (K, H)
    S[i, j] = sum_h w_v[h] * tanh(q[i,h] + k[j,h])
    out[b]  = softmax_j(S masked) @ values[b]

Sharding: the mask is a prefix mask (positions >= valid_len are dead), so
only sum(valid_lens) key columns carry work.  The host splits each batch's
valid-key prefix into jobs of KJ keys and hands each of the 8 cores NJ=2
jobs.  A job scores its KJ keys against all Q queries of its batch and
emits unnormalized partials (O^T = sum_j e^S v_j, l = sum_j e^S); the host
sums partials per batch and divides.  No max-subtraction is needed:
|S| <= sum|w_v| ~ 7, so exp never overflows in f32.

Device pipeline per key pair (2j, 2j+1), h in partitions:
  DVE  presum[:, :] = q2 + k2[:, j]      (tensor_scalar, per-partition addend)
  ACT  feats = tanh(presum)              (bulk over GP pairs, bf16 out)
  PE   S^T[2t:2t+2, :] += wpat_t^T @ feats   (zero-padded stationary lands
       each pair's scores at the right PSUM partitions -> S^T in key order)
  ACT  P = exp(S^T + mask_bias)          (prefix mask rides the bias)
  PE   O^T += V^T_block @ P ; l += 1^T @ P
"""

import sys

sys.path.insert(0, "/opt/trn_rl_repo")

import numpy as np

B, Q, KLEN, D_IN, H, D_V = 4, 1024, 1024, 256, 64, 128
NCORES = 8
NJ = 2  # jobs per core
MASK_VAL = -1.0e6
GP = 10  # key-pairs per bulk-tanh group

_CACHE = {}
LAST_RESULT = None


def _group_sizes(npairs, ramp_up, ramp_down):
    """Bulk-tanh group sizes: mostly GP, with small lead-in/lead-out groups at
    the kernel boundaries so ACT/PE pipeline fill+drain don't serialize (and
    the PE never idles past the HAM re-throttle window at the tail)."""
    up = [1, 1, 2, 4] if ramp_up else []
    down = [4, 2, 1, 1] if ramp_down else []
    mid = npairs - sum(up) - sum(down)
    if mid < 0:
        return [(2, "act")] * (npairs // 2) + [(1, "act")] * (npairs % 2)
    sizes = up + [GP] * (mid // GP) + ([mid % GP] if mid % GP else []) + down
    plan = []
    # 2-pair DVE-path (tanh addition formula) groups per block; fewer in the
    # final (ramp-down) block where the thinning pipeline can't hide them
    ndve = 2 if ramp_down else 5
    for s in sizes:
        if s == GP and ndve > 0:
            plan.append((s - 2, "act"))
            plan.append((2, "dve"))
            ndve -= 1
        else:
            plan.append((s, "act"))
    if ramp_up:
        # first two lead-in groups skip the DVE presum (bias-fused tanh):
        # shortens the kernel-start critical chain by the DVE hop
        plan[0] = (plan[0][0], "actb")
        plan[1] = (plan[1][0], "actb")
    assert sum(s for s, _ in plan) == npairs
    return plan


def _plan(vl):
    """Choose job size KJ and split batches' valid prefixes into NCORES*NJ jobs."""
    nslots = NCORES * NJ
    kj = 32
    while sum(-(-v // kj) for v in vl) > nslots:
        kj += 32
    jobs = []  # (batch, start, cnt)
    for b, v in enumerate(vl):
        nb_jobs = -(-v // kj)
        base, rem = divmod(v, nb_jobs)
        s = 0
        for i in range(nb_jobs):
            cnt = base + (1 if i < rem else 0)
            jobs.append((b, s, cnt))
            s += cnt
    while len(jobs) < nslots:
        jobs.append((0, 0, 0))  # empty padding job
    return kj, jobs


def _build(kj, repeat=1):
    import concourse.tile as tile
    from concourse import bacc, mybir

    fp32 = mybir.dt.float32
    bf16 = mybir.dt.bfloat16
    Tanh = mybir.ActivationFunctionType.Tanh
    Exp = mybir.ActivationFunctionType.Exp
    nbj = -(-kj // 128)  # key blocks per job
    hKJ = kj // 2

    nc = bacc.Bacc(
        "TRN2", target_bir_lowering=False, debug=False, num_devices=NCORES
    )
    qtsE = nc.dram_tensor("qts", [128, NJ * 2 * Q], bf16, kind="ExternalInput").ap()
    ktsE = nc.dram_tensor("kts", [128, NJ * 2 * kj], bf16, kind="ExternalInput").ap()
    vallE = nc.dram_tensor(
        "vall", [128, NJ * nbj * D_V], bf16, kind="ExternalInput"
    ).ap()
    mRE = nc.dram_tensor("maskR", [128, NJ * nbj], fp32, kind="ExternalInput").ap()
    wqE = nc.dram_tensor("wq", [128, 2 * H], bf16, kind="ExternalInput").ap()
    wkE = nc.dram_tensor("wk", [128, 2 * H], bf16, kind="ExternalInput").ap()
    wpE = nc.dram_tensor("wpat", [128, 512], bf16, kind="ExternalInput").ap()
    outE = nc.dram_tensor("out", [NJ * (D_V + 1), Q], fp32, kind="ExternalOutput").ap()

    with tile.TileContext(nc) as tc:
        with (
            tc.tile_pool(name="const", bufs=1) as cp,
            tc.tile_pool(name="feats", bufs=2) as fpool,
            tc.tile_pool(name="probs", bufs=2) as prp,
            tc.tile_pool(name="ps1", bufs=1, space="PSUM") as ps1,
            tc.tile_pool(name="ps2", bufs=2, space="PSUM") as ps2,
        ):
            # --- input DMAs: one contiguous transfer per tensor, spread over
            # both HWDGE rings (sync, scalar) + SWDGE (gpsimd); q-side first so
            # projections start early.
            qts = cp.tile([128, NJ * 2 * Q], bf16)
            for jn in range(NJ):
                nc.sync.dma_start(
                    qts[:, jn * 2 * Q : (jn + 1) * 2 * Q],
                    qtsE[:, jn * 2 * Q : (jn + 1) * 2 * Q],
                )
            wq_sb = cp.tile([128, 2 * H], bf16)
            nc.scalar.dma_start(wq_sb[:], wqE[:, :])
            wk_sb = cp.tile([128, 2 * H], bf16)
            nc.scalar.dma_start(wk_sb[:], wkE[:, :])
            kts = cp.tile([128, NJ * 2 * kj], bf16)
            nc.scalar.dma_start(kts[:], ktsE[:, :])
            wp_sb = cp.tile([128, 512], bf16)
            nc.gpsimd.dma_start(wp_sb[:], wpE[:, :])
            mask_sb = cp.tile([128, NJ * nbj], fp32)
            nc.gpsimd.dma_start(mask_sb[:], mRE[:, :])
            vall = cp.tile([128, NJ * nbj * D_V], bf16)
            nc.gpsimd.dma_start(vall[:], vallE[:, :])
            ones_sb = cp.tile([128, 1], bf16)
            nc.vector.memset(ones_sb[:], 1.0)

            o_sb = cp.tile([128, NJ * Q], fp32, name="o_sb")
            lo_sb = cp.tile([1, NJ * Q], fp32, name="lo_sb")

            def emit_proj(rep, jn):
                """Project one job's queries/keys; returns (q2, k2) SBUF tiles."""
                qof = jn * 2 * Q
                # q_proj^T: qh halves stacked in partitions of one bank
                qproj_ps = ps2.tile(
                    [128, 512], fp32, tag="S0", name=f"qproj_{rep}_{jn}", bufs=2
                )
                for qh in range(2):
                    for cc in range(2):
                        nc.tensor.matmul(
                            qproj_ps[64 * qh : 64 * qh + 64, :],
                            wq_sb[:, cc * H : (cc + 1) * H],
                            qts[:, qof + cc * Q + qh * 512 : qof + cc * Q + qh * 512 + 512],
                            start=(cc == 0),
                            stop=(cc == 1),
                        )
                q2_sb = cp.tile([128, Q], bf16, tag=f"q2_{jn}", name=f"q2_{rep}_{jn}")
                for qh in range(2):
                    if qh == 0 or jn > 0 or rep > 0:
                        nc.vector.tensor_copy(
                            q2_sb[0:64, qh * 512 : qh * 512 + 512],
                            qproj_ps[64 * qh : 64 * qh + 64, :],
                        )
                        nc.vector.tensor_copy(
                            q2_sb[64:128, qh * 512 : qh * 512 + 512],
                            qproj_ps[64 * qh : 64 * qh + 64, :],
                        )
                    else:
                        nc.scalar.copy(
                            q2_sb[0:64, qh * 512 : qh * 512 + 512],
                            qproj_ps[64 * qh : 64 * qh + 64, :],
                        )
                        nc.scalar.copy(
                            q2_sb[64:128, qh * 512 : qh * 512 + 512],
                            qproj_ps[64 * qh : 64 * qh + 64, :],
                        )
                # k2 = paired key projections: [:64] even keys, [64:] odd
                kof = jn * 2 * kj
                kproj_ps = ps2.tile(
                    [128, 512], fp32, tag="S1", name=f"kproj_{rep}_{jn}", bufs=2
                )
                for half in range(2):
                    for cc in range(2):
                        nc.tensor.matmul(
                            kproj_ps[64 * half : 64 * half + 64, 0:hKJ],
                            wk_sb[:, cc * H : (cc + 1) * H],
                            kts[:, kof + cc * kj + half * hKJ : kof + cc * kj + half * hKJ + hKJ],
                            start=(cc == 0),
                            stop=(cc == 1),
                        )
                k2_sb = cp.tile(
                    [128, hKJ], fp32, tag=f"k2_{jn}", name=f"k2_{rep}_{jn}"
                )
                nc.vector.tensor_copy(k2_sb[:], kproj_ps[:, 0:hKJ])
                return q2_sb, k2_sb

            for rep in range(repeat):
                # job 0's projections immediately; job 1's are emitted after
                # job 0's ramp-up groups (lower scheduler priority) so the
                # first tanh isn't stuck behind job 1's DVE copies.
                proj = {0: emit_proj(rep, 0)}

                for jn in range(NJ):
                    O_ps = [
                        ps1.tile(
                            [128, 512], fp32, tag=f"O{qh}", name=f"O{qh}_{rep}_{jn}"
                        )
                        for qh in range(2)
                    ]
                    l_ps = [
                        ps1.tile(
                            [1, 512], fp32, tag=f"l{qh}", name=f"l{qh}_{rep}_{jn}"
                        )
                        for qh in range(2)
                    ]
                    q2_sb, k2_sb = proj[jn]
                    tq2_sb = cp.tile(
                        [128, Q], fp32, tag=f"tq2_{jn}", name=f"tq2_{rep}_{jn}"
                    )
                    nc.scalar.activation(tq2_sb[:], q2_sb[:], Tanh)
                    tk2_sb = cp.tile(
                        [128, hKJ], fp32, tag=f"tk2_{jn}", name=f"tk2_{rep}_{jn}"
                    )
                    nc.scalar.activation(tk2_sb[:], k2_sb[:], Tanh)

                    # main loop over 128-key blocks of this job
                    for m in range(nbj):
                        kb = min(128, kj - m * 128)
                        npair_m = kb // 2
                        S_ps = [
                            ps2.tile(
                                [128, 512],
                                fp32,
                                tag=f"S{qh}",
                                name=f"S{qh}_{rep}_{jn}_{m}",
                                bufs=2,
                            )
                            for qh in range(2)
                        ]
                        gsizes = _group_sizes(
                            npair_m,
                            ramp_up=(rep == 0 and jn == 0 and m == 0),
                            ramp_down=(
                                rep == repeat - 1 and jn == NJ - 1 and m == nbj - 1
                            ),
                        )
                        tp0 = 0
                        for grp, (gp, eng) in enumerate(gsizes):
                            feats = fpool.tile(
                                [128, gp * Q],
                                bf16,
                                name=f"feats_{rep}_{jn}_{m}_{grp}",
                                tag="dfeats" if eng == "dve" else "feats",
                                bufs=2 if eng == "dve" else 3,
                            )
                            if eng == "dve":
                                # tanh(q+k) = (tq+tk)/(1+tq*tk), all on DVE
                                u_sb = fpool.tile(
                                    [128, gp * Q],
                                    fp32,
                                    name=f"u_{rep}_{jn}_{m}_{grp}",
                                    tag="upath",
                                    bufs=2,
                                )
                                for p in range(gp):
                                    j = 64 * m + tp0 + p
                                    nc.vector.tensor_scalar(
                                        u_sb[:, p * Q : (p + 1) * Q],
                                        tq2_sb[:],
                                        tk2_sb[:, j : j + 1],
                                        1.0,
                                        mybir.AluOpType.mult,
                                        mybir.AluOpType.add,
                                    )
                                nc.vector.reciprocal_approx_fast(
                                    u_sb[:, 0 : gp * Q], u_sb[:, 0 : gp * Q]
                                )
                                for p in range(gp):
                                    j = 64 * m + tp0 + p
                                    nc.vector.scalar_tensor_tensor(
                                        feats[:, p * Q : (p + 1) * Q],
                                        tq2_sb[:],
                                        tk2_sb[:, j : j + 1],
                                        u_sb[:, p * Q : (p + 1) * Q],
                                        mybir.AluOpType.add,
                                        mybir.AluOpType.mult,
                                    )
                            elif eng == "actb":
                                for p in range(gp):
                                    j = 64 * m + tp0 + p
                                    nc.scalar.activation(
                                        feats[:, p * Q : (p + 1) * Q],
                                        q2_sb[:],
                                        Tanh,
                                        bias=k2_sb[:, j : j + 1],
                                        scale=1.0,
                                    )
                            else:
                                presum = fpool.tile(
                                    [128, gp * Q],
                                    bf16,
                                    name=f"presum_{rep}_{jn}_{m}_{grp}",
                                    tag="presum",
                                    bufs=3,
                                )
                                for p in range(gp):
                                    j = 64 * m + tp0 + p
                                    nc.vector.tensor_scalar_add(
                                        presum[:, p * Q : (p + 1) * Q],
                                        q2_sb[:],
                                        k2_sb[:, j : j + 1],
                                    )
                                nc.scalar.activation(
                                    feats[:, 0 : gp * Q], presum[:, 0 : gp * Q], Tanh
                                )
                            for p in range(gp):
                                tp = tp0 + p
                                g, tl = divmod(tp, 16)
                                for qh in range(2):
                                    nc.tensor.matmul(
                                        S_ps[qh][32 * g : 32 * g + 32, :],
                                        wp_sb[:, 32 * tl : 32 * tl + 32],
                                        feats[:, p * Q + qh * 512 : p * Q + qh * 512 + 512],
                                        start=(tl == 0),
                                        stop=(tl == 15 or tp == npair_m - 1),
                                        tile_position=(0, 32 * g),
                                    )
                            tp0 += gp
                            if (
                                jn == 0
                                and m == 0
                                and grp == 6
                                and NJ > 1
                                and (jn + 1) not in proj
                            ):
                                proj[jn + 1] = emit_proj(rep, jn + 1)
                        if jn == 0 and m == 0 and NJ > 1 and 1 not in proj:
                            proj[1] = emit_proj(rep, 1)
                        for qh in range(2):
                            P_sb = prp.tile(
                                [128, 512],
                                bf16,
                                tag=f"P{qh}",
                                name=f"P{qh}_{rep}_{jn}_{m}",
                                bufs=2,
                            )
                            nc.scalar.activation(
                                P_sb[0:kb, :],
                                S_ps[qh][0:kb, :],
                                Exp,
                                bias=mask_sb[0:kb, jn * nbj + m : jn * nbj + m + 1],
                                scale=1.0,
                            )
                            nc.tensor.matmul(
                                O_ps[qh][:],
                                vall[0:kb, (jn * nbj + m) * D_V : (jn * nbj + m + 1) * D_V],
                                P_sb[0:kb, :],
                                start=(m == 0),
                                stop=(m == nbj - 1),
                            )
                            nc.tensor.matmul(
                                l_ps[qh][:],
                                ones_sb[0:kb, :],
                                P_sb[0:kb, :],
                                start=(m == 0),
                                stop=(m == nbj - 1),
                            )

                    for qh in range(2):
                        nc.vector.tensor_copy(
                            o_sb[:, jn * Q + qh * 512 : jn * Q + qh * 512 + 512],
                            O_ps[qh][:],
                        )
                        nc.vector.tensor_copy(
                            lo_sb[:, jn * Q + qh * 512 : jn * Q + qh * 512 + 512],
                            l_ps[qh][:],
                        )
                    if rep == repeat - 1:
                        nc.sync.dma_start(
                            outE[jn * (D_V + 1) : jn * (D_V + 1) + D_V, :],
                            o_sb[:, jn * Q : (jn + 1) * Q],
                        )
                        nc.sync.dma_start(
                            outE[jn * (D_V + 1) + D_V : jn * (D_V + 1) + D_V + 1, :],
                            lo_sb[:, jn * Q : (jn + 1) * Q],
                        )

    nc.compile()
    return nc


def _prepare(inputs):
    import ml_dtypes

    bf16 = ml_dtypes.bfloat16
    queries = np.asarray(inputs["queries"], dtype=np.float32)
    keys = np.asarray(inputs["keys"], dtype=np.float32)
    values = np.asarray(inputs["values"], dtype=np.float32)
    valid_lens = np.asarray(inputs["valid_lens"]).astype(np.int64)
    W_q = np.asarray(inputs["W_q"], dtype=np.float32)
    W_k = np.asarray(inputs["W_k"], dtype=np.float32)
    w_v = np.asarray(inputs["w_v"], dtype=np.float32)

    kj, jobs = _plan([int(x) for x in valid_lens])
    nbj = -(-kj // 128)

    wpat = np.zeros((128, 512), np.float32)
    for t in range(16):
        wpat[0:64, 32 * t + 2 * t] = w_v
        wpat[64:128, 32 * t + 2 * t + 1] = w_v
    wpat = wpat.astype(bf16)
    wq_r = np.concatenate([W_q[0:128], W_q[128:256]], axis=1).astype(bf16)
    wk_r = np.concatenate([W_k[0:128], W_k[128:256]], axis=1).astype(bf16)

    qT = {b: np.ascontiguousarray(queries[b].T) for b in range(B)}

    in_maps = []
    for c in range(NCORES):
        qts = np.empty((128, NJ * 2 * Q), bf16)
        kts = np.empty((128, NJ * 2 * kj), bf16)
        vall = np.zeros((128, NJ * nbj * D_V), bf16)
        maskR = np.full((128, NJ * nbj), MASK_VAL, np.float32)
        for jn in range(NJ):
            b, s, cnt = jobs[c * NJ + jn]
            qts[:, jn * 2 * Q : jn * 2 * Q + Q] = qT[b][0:128].astype(bf16)
            qts[:, jn * 2 * Q + Q : (jn + 1) * 2 * Q] = qT[b][128:256].astype(bf16)
            kp = np.zeros((kj, D_IN), np.float32)
            kp[0:cnt] = keys[b, s : s + cnt]
            kre = np.concatenate([kp[0::2], kp[1::2]], axis=0).T  # (256, kj)
            kts[:, jn * 2 * kj : jn * 2 * kj + kj] = kre[0:128].astype(bf16)
            kts[:, jn * 2 * kj + kj : (jn + 1) * 2 * kj] = kre[128:256].astype(bf16)
            vp = np.zeros((kj, D_V), np.float32)
            vp[0:cnt] = values[b, s : s + cnt]
            for m in range(nbj):
                kb = min(128, kj - m * 128)
                vall[0:kb, (jn * nbj + m) * D_V : (jn * nbj + m) * D_V + D_V] = vp[
                    m * 128 : m * 128 + kb
                ].astype(bf16)
                mm = np.full((128,), MASK_VAL, np.float32)
                nvalid = min(max(cnt - m * 128, 0), 128)
                mm[0:nvalid] = 0.0
                maskR[:, jn * nbj + m] = mm
        in_maps.append(
            {
                "qts": qts,
                "kts": kts,
                "vall": vall,
                "maskR": maskR,
                "wq": wq_r,
                "wk": wk_r,
                "wpat": wpat,
            }
        )
    return kj, jobs, in_maps


def kernel(**inputs):
    global LAST_RESULT
    kj, jobs, in_maps = _prepare(inputs)

    if kj not in _CACHE:
        _CACHE[kj] = _build(kj)
    nc = _CACHE[kj]

    from concourse.bass_utils import run_bass_kernel_spmd

    res = run_bass_kernel_spmd(nc, in_maps, core_ids=list(range(NCORES)))
    LAST_RESULT = res

    O = np.zeros((B, D_V, Q), np.float64)
    L = np.zeros((B, Q), np.float64)
    for c in range(NCORES):
        o = np.asarray(res.results[c]["out"])  # (NJ*(D_V+1), Q)
        for jn in range(NJ):
            b, s, cnt = jobs[c * NJ + jn]
            if cnt == 0:
                continue
            O[b] += o[jn * (D_V + 1) : jn * (D_V + 1) + D_V].astype(np.float64)
            L[b] += o[jn * (D_V + 1) + D_V].astype(np.float64)
    out = (O / L[:, None, :]).transpose(0, 2, 1)
    return np.ascontiguousarray(out.astype(np.float32))
